# revision 3
# baseline (speedup 1.0000x reference)
"""Binarized 3-layer MLP on 8 TRN2 NeuronCores (data-parallel over batch).

Computation (matching the reference):
    h1  = x @ sign(W1).T          x: [65536, 784] fp32, W1: [400, 784]
    h2  = sign(h1) @ sign(W2).T   W2: [200, 400]
    out = sign(h2) @ sign(W3).T   W3: [10, 200]

v2 strategy (delta over the fp16 hi/lo baseline):
  - Layer-1 x precision: hi = fp16(x) (exact fp16 matmuls), residual
    lo = (x - hi) * 2^12 quantized to fp8 e4m3, contracted with e5m2
    weights (+-2^-12) so the PSUM accumulates hi + lo*2^-12 ~ fp32 x to
    ~2^-15 relative. Measured end-to-end rel err 0.007 (gate 2e-2).
  - The 6 full lo k-tiles run as fp8 DoubleRow matmuls (2 k-tiles per
    instruction, 2 fp8 MACs/cell/cycle): 3 DR MMs replace 6 full MMs.
    K-rows 768:784 (hi + raw fp16 residual) ride in one 32-row fp16
    tail subtile, row-strip-packed so the three m-tiles' tails run
    concurrently.
  - Layer 2: features 0:256 as one fp8 DoubleRow MM (sign values are
    exact in e4m3); features 256:384 stay bf16; 384:400 packed strips.
  - Batch sharded 8192 rows/core; weights replicated. m4 (h1 rows
    384:400) packed into one PSUM bank via col-strips as in baseline.
  - fp16<->DoubleRow perf-mode switches drain the PE pipe (~190ns), so
    DR matmuls are batched per chunk (2 switches instead of 6).
  - Startup: first chunk's x DMA split per-k-subtile with its first
    subtile's trigger leading every engine queue; PE warmup matmuls
    bridge the HAM activity window until data lands. Out-DMA triggers
    spread across sync/gpsimd/scalar to shorten the tail.
"""

import contextlib
import ctypes
import os
import sys
import types

import numpy as np
import ml_dtypes

import concourse.bacc as bacc
import concourse.mybir as mybir
import concourse.tile as tile
from concourse.bass_utils import run_bass_kernel_spmd


def _ensure_axon_hooks():
    """concourse's trace path imports antenv.axon_hooks, which this image
    lacks; register a ctypes-backed stand-in so trace=True (or a stray
    BASS_TRACE=1 in the environment) cannot crash the run."""
    try:
        import antenv.axon_hooks  # noqa: F401
        return
    except ImportError:
        pass

    so_path = "/opt/axon/libaxon_pjrt.so"
    hook = None
    if os.path.exists(so_path):
        try:
            lib = ctypes.CDLL(so_path)
            if hasattr(lib, "axon_start_nrt_profile"):
                lib.axon_start_nrt_profile.argtypes = [
                    ctypes.POINTER(ctypes.c_int64),
                    ctypes.c_size_t,
                ]
                lib.axon_start_nrt_profile.restype = ctypes.c_int64
                lib.axon_stop_nrt_profile.argtypes = [ctypes.c_char_p]
                lib.axon_stop_nrt_profile.restype = ctypes.c_int64

                @contextlib.contextmanager
                def _hook(output_dir, device_ids):
                    import jax

                    jax.devices()
                    if device_ids:
                        ids = (ctypes.c_int64 * len(device_ids))(*device_ids)
                        rc = lib.axon_start_nrt_profile(ids, len(device_ids))
                    else:
                        rc = lib.axon_start_nrt_profile(None, 0)
                    if rc != 0:
                        raise RuntimeError(f"axon_start_nrt_profile rc={rc}")
                    try:
                        yield
                    finally:
                        lib.axon_stop_nrt_profile(str(output_dir).encode())

                hook = _hook
        except OSError:
            pass

    mod = types.ModuleType("antenv.axon_hooks")
    mod.get_axon_ntff_profile_hook = lambda: hook
    mod.set_axon_ntff_profile_hook = lambda h: None
    sys.modules["antenv.axon_hooks"] = mod

    import concourse.bass_utils as _bu

    _bu.upload_artifacts = lambda tmpdir: tmpdir


BF16 = np.dtype(ml_dtypes.bfloat16)
E4 = np.dtype(ml_dtypes.float8_e4m3)
E5 = np.dtype(ml_dtypes.float8_e5m2)

NCORES = 8
B = 65536
BL = B // NCORES          # 8192 rows per core
D0, H1, H2, DO = 784, 400, 200, 10
CH = 512                  # batch columns per chunk (PSUM bank = 512 fp32)
NCH = BL // CH            # 16 chunks per core
GRP = 4                   # chunks per packing group
H2P = 208                 # w2dr padded cols (DoubleRow needs 16B-aligned steps)
KH = 6                    # full 128-row k-subtiles (768 of 784 K rows)
LOSC = 12                 # lo scale exponent: x-side *2^12, w-side 2^-12

_cache = {}


def _build():
    if "nc" in _cache:
        return _cache["nc"]

    f32 = mybir.dt.float32
    bf16 = mybir.dt.bfloat16
    f16 = mybir.dt.float16
    e4 = mybir.dt.float8e4
    e5 = mybir.dt.float8e5
    DRM = mybir.MatmulPerfMode.DoubleRow
    Sign = mybir.ActivationFunctionType.Sign

    nc = bacc.Bacc("TRN2", debug=False, num_devices=NCORES)

    # x: hi fp16 [128, 7, CH] per chunk (subtile 6 = 32-row K-tail packed
    # at strips 0/32/64), lo e4m3 [128, 6, CH] per chunk
    d_xhi = nc.dram_tensor("xhi", [NCH, 128, KH + 1, CH], f16, kind="ExternalInput").ap()
    d_xlo = nc.dram_tensor("xlo", [NCH, 128, KH, CH], e4, kind="ExternalInput").ap()
    # w1 hi split so the first m-slab lands before the rest
    d_w1ha = nc.dram_tensor("w1ha", [128, KH + 1, 128], f16, kind="ExternalInput").ap()
    d_w1hb = nc.dram_tensor("w1hb", [128, KH + 1, H1 - 128], f16, kind="ExternalInput").ap()
    d_w1lo = nc.dram_tensor("w1lo", [128, KH, H1], e5, kind="ExternalInput").ap()
    d_w2dr = nc.dram_tensor("w2dr", [128, 2, H2P], e4, kind="ExternalInput").ap()
    # w2b: [:,0,:] features 256:384 bf16; [:,1,:] features 384:400 packed
    # at partition strips 32jj..32jj+16
    d_w2b = nc.dram_tensor("w2b", [128, 2, H2], bf16, kind="ExternalInput").ap()
    d_w3 = nc.dram_tensor("w3", [128, 2, DO], bf16, kind="ExternalInput").ap()
    d_out = nc.dram_tensor("out", [NCH, DO, CH], f32, kind="ExternalOutput").ap()

    m2sz = [128, 72]
    k3sz = [128, 72]

    with tile.TileContext(nc) as tc:
        with (
            tc.tile_pool(name="wp", bufs=1) as wp,
            tc.tile_pool(name="xhp", bufs=8) as xhp,
            tc.tile_pool(name="xlp", bufs=8) as xlp,
            tc.tile_pool(name="adrp", bufs=2) as adrp,
            tc.tile_pool(name="am2p", bufs=2) as am2p,
            tc.tile_pool(name="a13pp", bufs=2) as a13pp,
            tc.tile_pool(name="a2p", bufs=2) as a2pool,
            tc.tile_pool(name="op", bufs=2) as op,
            tc.tile_pool(name="ps1p", bufs=1, space="PSUM") as ps1p,
            tc.tile_pool(name="ps2p", bufs=1, space="PSUM") as ps2p,
            tc.tile_pool(name="pspk", bufs=2, space="PSUM") as pspk,
        ):
            w1ha = wp.tile([128, KH + 1, 128], f16, name="w1ha")
            w1hb = wp.tile([128, KH + 1, H1 - 128], f16, name="w1hb")
            w1lo = wp.tile([128, KH, H1], e5, name="w1lo")
            w2dr = wp.tile([128, 2, H2P], e4, name="w2dr")
            w2b = wp.tile([128, 2, H2], bf16, name="w2b")
            w3sb = wp.tile([128, 2, DO], bf16, name="w3sb")

            # critical first loads: first MM needs xhi chunk-0 subtile 0 and
            # w1ha k0; issue those at the very front of each engine queue so
            # they hit the DMA engines first (bandwidth is shared round-robin)
            xhc0 = xhp.tile([128, KH + 1, CH], f16, name="xhc")
            xlc0 = xlp.tile([128, KH, CH], e4, name="xlc")
            nc.sync.dma_start(out=xhc0[:, 0:1, :], in_=d_xhi[0][:, 0:1, :])
            nc.gpsimd.dma_start(out=w1ha[:, 0:2, :], in_=d_w1ha[:, 0:2, :])
            nc.scalar.dma_start(out=xhc0[:, 1:4, :], in_=d_xhi[0][:, 1:4, :])
            nc.scalar.dma_start(out=xhc0[:, 4:7, :], in_=d_xhi[0][:, 4:7, :])
            nc.gpsimd.dma_start(out=w1ha[:, 2:7, :], in_=d_w1ha[:, 2:7, :])
            nc.gpsimd.dma_start(out=xlc0[:], in_=d_xlo[0])
            nc.sync.dma_start(out=w1hb[:], in_=d_w1hb)
            nc.gpsimd.dma_start(out=w1lo[:], in_=d_w1lo)

            def w1h_slice(k, m_off, m_sz):
                if m_off == 0:
                    return w1ha[:, k, 0:m_sz]
                return w1hb[:, k, m_off - 128 : m_off - 128 + m_sz]

            def w1lo_slice(t, m_off, m_sz):
                return w1lo[:, 2 * t : 2 * t + 2, m_off : m_off + m_sz]

            def layer1_m123(xhc, xlc):
                """Full-width layer-1 m-tiles; returns [a1dr, a1m2].

                Per m: 6 fp16 hi MMs + 3 fp8 DoubleRow lo MMs; the 32-row
                fp16 tail (hi+lo of K rows 768:784, replicated at strips
                0/32/64) closes each group concurrently."""
                pss = []
                for m in range(3):
                    ps = ps1p.tile(
                        [128, CH], f32, name=f"ps1_{m}", bufs=(2 if m == 0 else 1)
                    )
                    for k in range(KH):
                        nc.tensor.matmul(
                            ps[:],
                            w1h_slice(k, m * 128, 128),
                            xhc[:, k, :],
                            start=(k == 0),
                            stop=False,
                        )
                    pss.append(ps)
                # DR matmuls batched after all fp16 (normal<->DR perf-mode
                # switches drain the PE pipe, ~190ns each)
                for m in range(3):
                    for t in range(KH // 2):
                        nc.tensor.matmul(
                            pss[m][:],
                            w1lo_slice(t, m * 128, 128),
                            xlc[:, 2 * t : 2 * t + 2, :],
                            start=False,
                            stop=False,
                            perf_mode=DRM,
                        )
                kl = KH
                for m in range(3):
                    s = 32 * m
                    lhsT = (
                        w1ha[s : s + 32, kl, 0:128]
                        if m == 0
                        else w1hb[s : s + 32, kl, (m - 1) * 128 : m * 128]
                    )
                    nc.tensor.matmul(
                        pss[m][:],
                        lhsT,
                        xhc[s : s + 32, kl, :],
                        start=False,
                        stop=True,
                        tile_position=(s, 0),
                    )
                a1dr = adrp.tile([128, 2, CH], e4, name="a1dr")
                a1m2 = am2p.tile([128, CH], bf16, name="a1m2")
                nc.scalar.activation(a1dr[:, 0, :], pss[0][:], Sign)
                nc.scalar.activation(a1dr[:, 1, :], pss[1][:], Sign)
                nc.scalar.activation(a1m2[:], pss[2][:], Sign)
                return [a1dr, a1m2]

            def layer2(jj, a1m, a13p):
                """Layer 2 for chunk jj of the group. Returns [a2_m0, a2_m1]."""
                a1dr, a1m2 = a1m
                a2 = [None, None]
                order = (0, 1) if jj % 2 == 0 else (1, 0)
                pss = {}
                s = 32 * jj
                for m in order:
                    sz = m2sz[m]
                    ps = ps2p.tile([sz, CH], f32, name=f"ps2_{m}")
                    nc.tensor.matmul(
                        ps[:],
                        w2dr[:, :, m * 128 : m * 128 + sz],
                        a1dr[:],
                        start=True,
                        stop=False,
                        perf_mode=DRM,
                    )
                    pss[m] = ps
                for m in order:
                    sz = m2sz[m]
                    nc.tensor.matmul(
                        pss[m][:],
                        w2b[:, 0, m * 128 : m * 128 + sz],
                        a1m2[:],
                        start=False,
                        stop=False,
                    )
                for m in order:
                    sz = m2sz[m]
                    nc.tensor.matmul(
                        pss[m][:],
                        w2b[s : s + 16, 1, m * 128 : m * 128 + sz],
                        a13p[s : s + 16, :],
                        start=False,
                        stop=True,
                        tile_position=(s, 0),
                    )
                for m in order:
                    at = a2pool.tile([m2sz[m], CH], bf16, name=f"a2_{jj}_{m}")
                    nc.scalar.activation(at[:], pss[m][:], Sign)
                    a2[m] = at
                return a2

            # HAM/P-state pre-warm: N=512 dummy matmuls (426ns cold, 216ns
            # warm) keep the PE busy and the HAM activity window filled until
            # the first x data lands (~15.5us: engine preamble ~7us +
            # shared-bandwidth DMA ~8us); ~30 covers the span with little
            # leftover if data arrives early.
            warm = wp.tile([128, 512], f16, name="warm")
            nc.vector.memset(warm[:], 1.0)
            wps = pspk.tile([64, 512], f32, name="wps", tag="pack")
            for _ in range(30):
                nc.tensor.matmul(wps[:], warm[:, 0:64], warm[:], start=True, stop=True)

            for g in range(NCH // GRP):
                xhs = []
                xls = []
                for jj in range(GRP):
                    ci = g * GRP + jj
                    if g == 0 and jj == 0:
                        xhs.append(xhc0)
                        xls.append(xlc0)
                        continue
                    xhc = xhp.tile([128, KH + 1, CH], f16, name="xhc")
                    xlc = xlp.tile([128, KH, CH], e4, name="xlc")
                    nc.sync.dma_start(out=xhc[:], in_=d_xhi[ci])
                    nc.gpsimd.dma_start(out=xlc[:], in_=d_xlo[ci])
                    xhs.append(xhc)
                    xls.append(xlc)
                    if g == 0 and jj == 1:
                        nc.scalar.dma_start(out=w2dr[:], in_=d_w2dr)
                        nc.scalar.dma_start(out=w2b[:], in_=d_w2b)
                        nc.scalar.dma_start(out=w3sb[:], in_=d_w3)

                # packed m4 PSUM bank: strips [32jj : 32jj+16] per chunk
                ps4 = pspk.tile([128, CH], f32, name="ps4", tag="pack")
                nc.vector.memset(ps4[:], 0.0)

                a1s = [None] * GRP
                a1s[0] = layer1_m123(xhs[0], xls[0])
                a1s[1] = layer1_m123(xhs[1], xls[1])

                # m4 packed: hi k-subtiles + fp16 tail + fp8 lo, col strips
                for k in range(KH + 1):
                    kr = 32 if k == KH else 128
                    for jj in range(GRP):
                        s = 32 * jj
                        nc.tensor.matmul(
                            ps4[s : s + 16, :],
                            w1h_slice(k, 384, 16)[0:kr],
                            xhs[jj][0:kr, k, :],
                            start=False,
                            stop=False,
                            tile_position=(0, s),
                        )
                for k in range(KH):
                    for jj in range(GRP):
                        s = 32 * jj
                        nc.tensor.matmul(
                            ps4[s : s + 16, :],
                            w1lo[:, k, 384:400],
                            xls[jj][:, k, :],
                            start=False,
                            stop=(k == KH - 1),
                            tile_position=(0, s),
                        )
                a13p = a13pp.tile([128, CH], bf16, name="a13p")
                nc.scalar.activation(a13p[:], ps4[:], Sign)

                a2s = [None] * GRP
                a2s[0] = layer2(0, a1s[0], a13p)
                a2s[1] = layer2(1, a1s[1], a13p)
                a1s[2] = layer1_m123(xhs[2], xls[2])
                a1s[3] = layer1_m123(xhs[3], xls[3])
                a2s[2] = layer2(2, a1s[2], a13p)
                a2s[3] = layer2(3, a1s[3], a13p)

                # layer 3, packed into one PSUM bank at strips [32jj:32jj+10]
                ps3 = pspk.tile([128, CH], f32, name="ps3", tag="pack")
                nc.vector.memset(ps3[:], 0.0)
                for k in range(2):
                    ks = k3sz[k]
                    for jj in range(GRP):
                        s = 32 * jj
                        nc.tensor.matmul(
                            ps3[s : s + DO, :],
                            w3sb[0:ks, k, :],
                            a2s[jj][k][0:ks, :],
                            start=False,
                            stop=(k == 1),
                            tile_position=(0, s),
                        )
                osb = op.tile([128, CH], f32, name="osb")
                nc.vector.tensor_copy(osb[:], ps3[:])
                # spread the 4 out triggers over engines (tail latency)
                outeng = [nc.sync, nc.gpsimd, nc.scalar, nc.sync]
                for jj in range(GRP):
                    s = 32 * jj
                    outeng[jj].dma_start(
                        out=d_out[g * GRP + jj], in_=osb[s : s + DO, :]
                    )

    nc.compile()
    _cache["nc"] = nc
    return nc


def _prep_weights(W1, W2, W3):
    # [K, M] layouts, K on partitions.
    w1T = np.sign(W1).T.astype(np.float32)  # [784, 400]
    w1h = np.zeros((128, KH + 1, H1), np.float32)
    for k in range(KH):
        w1h[:, k, :] = w1T[k * 128 : (k + 1) * 128]
    # tail subtile: hi rows 768:784 at 0:16, lo rows at 16:32 (same +-1
    # signs; the x side carries hi vs residual), replicated at strips
    w1h[0:16, KH, :] = w1T[768:784]
    w1h[16:32, KH, :] = w1T[768:784]
    w1h[32:64, KH, :] = w1h[0:32, KH, :]
    w1h[64:96, KH, :] = w1h[0:32, KH, :]
    w1ha = np.ascontiguousarray(w1h[:, :, 0:128]).astype(np.float16)
    w1hb = np.ascontiguousarray(w1h[:, :, 128:H1]).astype(np.float16)

    w1lo = np.zeros((128, KH, H1), np.float32)
    for k in range(KH):
        w1lo[:, k, :] = w1T[k * 128 : (k + 1) * 128] * 2.0**-LOSC
    w1lo = w1lo.astype(E5)

    w2T = np.sign(W2).T.astype(np.float32)  # [400, 200]
    w2dr = np.zeros((128, 2, H2P), np.float32)
    w2dr[:, 0, 0:H2] = w2T[0:128]
    w2dr[:, 1, 0:H2] = w2T[128:256]
    w2dr = w2dr.astype(E4)
    w2b = np.zeros((128, 2, H2), np.float32)
    w2b[:, 0, :] = w2T[256:384]
    for jj in range(GRP):
        w2b[32 * jj : 32 * jj + 16, 1, :] = w2T[384:400]
    w2b = w2b.astype(BF16)

    w3T = np.sign(W3).T.astype(np.float32)  # [200, 10]
    w3h = np.zeros((128, 2, DO), np.float32)
    w3h[:, 0, :] = w3T[0:128]
    w3h[0:72, 1, :] = w3T[128:200]
    w3h = w3h.astype(BF16)
    return w1ha, w1hb, w1lo, w2dr, w2b, w3h


def _prep_x_core(xc):
    # xc: [8192, 784] fp32 -> xhi [16, 128, 7, 512] f16, xlo [16, 128, 6, 512] e4
    xt = np.ascontiguousarray(xc.T.astype(np.float32))  # [784, 8192]
    hi = xt.astype(np.float16)
    res = xt - hi.astype(np.float32)
    xhi = np.zeros((128, KH + 1, BL), np.float16)
    for k in range(KH):
        xhi[:, k, :] = hi[k * 128 : (k + 1) * 128]
    xhi[0:16, KH, :] = hi[768:784]
    xhi[16:32, KH, :] = res[768:784].astype(np.float16)
    xhi[32:64, KH, :] = xhi[0:32, KH, :]
    xhi[64:96, KH, :] = xhi[0:32, KH, :]
    xhi = np.ascontiguousarray(
        xhi.reshape(128, KH + 1, NCH, CH).transpose(2, 0, 1, 3)
    )  # [16, 128, 7, 512]
    xlo = (res[0:768] * 2.0**LOSC).astype(E4)  # [768, 8192]
    xlo = np.ascontiguousarray(
        xlo.reshape(KH, 128, NCH, CH).transpose(2, 1, 0, 3)
    )  # [16, 128, 6, 512]
    return xhi, xlo


def kernel(x, W1, W2, W3, _trace=False, **_kw):
    nc = _build()
    w1ha, w1hb, w1lo, w2dr, w2b, w3h = _prep_weights(
        np.asarray(W1, np.float32), np.asarray(W2, np.float32), np.asarray(W3, np.float32)
    )
    x = np.asarray(x, np.float32).reshape(B, D0)

    in_maps = []
    for c in range(NCORES):
        xhi, xlo = _prep_x_core(x[c * BL : (c + 1) * BL])
        in_maps.append(
            {
                "xhi": xhi,
                "xlo": xlo,
                "w1ha": w1ha,
                "w1hb": w1hb,
                "w1lo": w1lo,
                "w2dr": w2dr,
                "w2b": w2b,
                "w3": w3h,
            }
        )

    _ensure_axon_hooks()
    res = run_bass_kernel_spmd(nc, in_maps, core_ids=list(range(NCORES)), trace=_trace)

    out = np.empty((B, DO), np.float32)
    for c in range(NCORES):
        oc = res.results[c]["out"]  # [16, 10, 512]
        out[c * BL : (c + 1) * BL] = oc.transpose(0, 2, 1).reshape(BL, DO)
    if _trace:
        _cache["last_results"] = res
    return out
